# revision 11
# baseline (speedup 1.0000x reference)
"""Bass/Tile kernel for the sharded NT-Xent contrastive loss (streaming fp8).

Per-core computation (core c of 8), B=8192 D=512 M=1024:
  inputs (host pre-transposed, layout only):
    vt [512, 1024] f32 : v[c*M:(c+1)*M, :].T   (this core's v-shard, transposed)
    ut [512, 8192] f32 : u.T                    (full u, transposed)
    us [512, 1024] f32 : u[c*M:(c+1)*M, :].T   (u-shard cols, for the diagonal)
  output:
    loss [1024] f32 : loss rows c*M:(c+1)*M

  math:
    invu[j] = 1/||u_j||  computed as exp(-0.5*ln(ss_u[j]) + ln 16)  (x16 scale
              keeps the fp8 u entries in the normal range)
    un16 = u * invu16  (fp8e4)      vb = v cast fp8e4 (NOT normalized)
    S[i,j] = sum_d vb[d,i]*un16[d,j]   (PE DoubleRow fp8, psum f32, = 16*||v_i||*cos)
    den[i] = sum_j exp((2/(16*||v_i||)) * S[i,j])   (ACT exp accum_out)
    e_in[i] = 2*cos(v_i,u_i) ; loss[i] = ln(exp(e_in) + den) - e_in

u is streamed in column chunks; the chunk norm pipeline (square on Pool,
ones-colsum on PE, compact rsqrt on ACT via ln/exp — single ACT table for the
whole kernel — and a stride-0 broadcast DMA) runs 1-2 chunks ahead of the
matmul+exp pipeline so the ACT engine (the roofline bottleneck at ~55us of
exp work) never idles.
"""

from contextlib import ExitStack
from math import log

import concourse.bass as bass
import concourse.tile as tile
from concourse import masks
from concourse import bacc, mybir

F32 = mybir.dt.float32
BF16 = mybir.dt.bfloat16
FP8 = mybir.dt.float8e4
MULT = mybir.AluOpType.mult
ADD = mybir.AluOpType.add
SUB = mybir.AluOpType.subtract
AF = mybir.ActivationFunctionType
DR = mybir.MatmulPerfMode.DoubleRow

B = 8192
D = 512
NCORES = 8
M = B // NCORES   # 1024
KT = D // 128     # 4 d-tiles
NIT = M // 128    # 8 i-tiles
LN16 = log(16.0)

CH = [1024, 1024, 2048, 2048, 2048]          # u column chunk sizes
OFF = [0, 1024, 2048, 4096, 6144]            # chunk column offsets
NC_CH = len(CH)


def build_nc():
    nc = bacc.Bacc("TRN2", target_bir_lowering=False, debug=False,
                   num_devices=NCORES)

    vt = nc.dram_tensor("vt", [D, M], F32, kind="ExternalInput")
    ut = nc.dram_tensor("ut", [D, B], F32, kind="ExternalInput")
    us = nc.dram_tensor("us", [D, M], F32, kind="ExternalInput")
    loss = nc.dram_tensor("loss", [M], F32, kind="ExternalOutput")
    # DRAM bounce buffers for transposes / broadcasts
    b_ss = nc.dram_tensor("b_ss", [B], BF16)     # u sum-of-squares rows
    b_inv = nc.dram_tensor("b_inv", [B], FP8)    # 16/||u_j|| flat
    b_v = nc.dram_tensor("b_v", [M], F32)       # v ss row
    b_us = nc.dram_tensor("b_us", [M], F32)     # u-shard ss row
    b_dot = nc.dram_tensor("b_dot", [M], F32)   # v.u dot row

    with tile.TileContext(nc) as tc, ExitStack() as ctx:
        consts = ctx.enter_context(tc.tile_pool(name="consts", bufs=1))
        keep = ctx.enter_context(tc.tile_pool(name="keep", bufs=1))
        upool = ctx.enter_context(tc.tile_pool(name="upool", bufs=1))
        stage = ctx.enter_context(tc.tile_pool(name="stage", bufs=2))
        vst = ctx.enter_context(tc.tile_pool(name="vst", bufs=1))
        sqp = ctx.enter_context(tc.tile_pool(name="sqp", bufs=2))
        rowp = ctx.enter_context(tc.tile_pool(name="rowp", bufs=2))
        invp = ctx.enter_context(tc.tile_pool(name="invp", bufs=2))
        expool = ctx.enter_context(tc.tile_pool(name="expool", bufs=1))
        mps = ctx.enter_context(tc.tile_pool(name="mps", bufs=2, space="PSUM"))

        # all-ones fp8 stationary operand: DoubleRow colsum of rhs
        ones8 = consts.tile([128, 2, 128], FP8)
        nc.vector.memset(ones8[:], 1.0)
        bias16 = consts.tile([128, 1], F32)
        nc.vector.memset(bias16[:], LN16)
        id8 = consts.tile([8, 8], F32)
        masks.make_identity(nc, id8[:])
        id128 = consts.tile([128, 128], F32)
        masks.make_identity(nc, id128[:])

        # persistent tiles
        un = [keep.tile([128, 2, B], FP8, tag=f"un{p}", name=f"un{p}")
              for p in range(2)]                 # normalized*16 u, kpair-packed
        vb = [keep.tile([128, 2, M], FP8, tag=f"vb{p}", name=f"vb{p}")
              for p in range(2)]                 # raw v fp8, kpair-packed
        inv2v16 = keep.tile([128, NIT], F32)     # 2/(16*||v_i||)
        e_in = keep.tile([128, NIT], F32)        # 2*cos(v_i, u_i)
        dpm = [keep.tile([128, NIT], F32, tag=f"dpm{k}", name=f"dpm{k}")
               for k in range(NC_CH)]            # per-chunk exp row sums
        invv_t = keep.tile([128, NIT], F32)
        invus_t = keep.tile([128, NIT], F32)
        ssv_t = keep.tile([128, NIT], F32)
        ssus_t = keep.tile([128, NIT], F32)
        dot_t = keep.tile([128, NIT], F32)
        lssv = keep.tile([128, NIT], F32)
        lssus = keep.tile([128, NIT], F32)

        vstage = [vst.tile([128, M], F32, tag=f"vs{dt}", name=f"vs{dt}")
                  for dt in range(KT)]
        usstage = [vst.tile([128, M], F32, tag=f"us{dt}", name=f"us{dt}")
                   for dt in range(KT)]

        # ---------------- emission helpers ----------------
        def dma_chunk(k):
            c, j0 = CH[k], OFF[k]
            for dt in range(KT):
                t = stage.tile([128, 2048], F32, tag=f"ust{dt}", bufs=3)
                nc.sync.dma_start(
                    t[:, :c], ut.ap()[dt * 128:(dt + 1) * 128, j0:j0 + c])
                stage_t[k][dt] = t

        stage_t = [[None] * KT for _ in range(NC_CH)]
        sc_t = [None] * NC_CH    # compact ss [128, 16] bf16 per chunk
        invc_t = [None] * NC_CH  # compact invu16 fp8 per chunk

        def norm_a(k):
            """square -> colsum -> row bounce -> compact ss (no ACT)."""
            c, j0 = CH[k], OFF[k]
            sq = [sqp.tile([128, 2, 2048], FP8, tag=f"usq{p}", name=f"usq{p}", bufs=1) for p in range(2)]
            for dt in range(KT):
                st = stage_t[k][dt]
                nc.gpsimd.tensor_tensor(sq[dt // 2][:, dt % 2, :c],
                                        st[:, :c], st[:, :c], MULT)
            ps = mps.tile([128, 2048], F32, tag="mm")
            for jc in range(c // 512):
                for p in range(2):
                    nc.tensor.matmul(
                        ps[:, jc * 512:(jc + 1) * 512], lhsT=ones8[:],
                        rhs=sq[p][:, :, jc * 512:(jc + 1) * 512],
                        start=(p == 0), stop=(p == 1), perf_mode=DR)
            fl = rowp.tile([1, 2048], BF16, tag="flss", bufs=1)
            nc.vector.tensor_copy(fl[:, :c], ps[0:1, :c])
            nc.sync.dma_start(b_ss.ap()[j0:j0 + c], fl[:, :c])
            # row-major compact [w, 128]: contiguous per partition, w descriptors
            sc = rowp.tile([16, 128], BF16, tag="scss")
            nc.sync.dma_start(
                sc[:c // 128, :],
                b_ss.ap()[j0:j0 + c].rearrange("(t p) -> t p", p=128))
            sc_t[k] = sc

        def norm_b(k):
            """compact rsqrt (ACT) -> flat bounce -> bcast dma -> fp8 cast."""
            c, j0 = CH[k], OFF[k]
            w = c // 128
            sc = sc_t[k]
            lss = rowp.tile([16, 128], F32, tag="lnss")
            nc.scalar.activation(lss[:w, :], sc[:w, :], AF.Ln)
            invc = rowp.tile([16, 128], FP8, tag="invc")
            nc.scalar.activation(invc[:w, :], lss[:w, :], AF.Exp,
                                 scale=-0.5, bias=bias16[:w])
            nc.sync.dma_start(
                b_inv.ap()[j0:j0 + c].rearrange("(t p) -> t p", p=128),
                invc[:w, :])
            invu = invp.tile([128, 2048], FP8, tag="invu", bufs=1)
            nc.sync.dma_start(
                invu[:, :c],
                b_inv.ap()[j0:j0 + c].partition_broadcast(128))
            for dt in range(KT):
                nc.vector.tensor_tensor(
                    un[dt // 2][:, dt % 2, j0:j0 + c],
                    stage_t[k][dt][:, :c], invu[:, :c], MULT)

        def it_group(k, it):
            c, j0 = CH[k], OFF[k]
            ps = mps.tile([128, 2048], F32, tag="mm")
            for p in range(2):
                for jc in range(c // 512):
                    nc.tensor.matmul(
                        ps[:, jc * 512:(jc + 1) * 512],
                        lhsT=vb[p][:, :, it * 128:(it + 1) * 128],
                        rhs=un[p][:, :, j0 + jc * 512:j0 + (jc + 1) * 512],
                        start=(p == 0), stop=(p == 1), perf_mode=DR)
            ex = expool.tile([128, 2048], FP8, tag="ex")
            nc.scalar.activation(ex[:, :c], ps[:, :c], AF.Exp,
                                 scale=inv2v16[:, it:it + 1],
                                 accum_out=dpm[k][:, it:it + 1])

        def colsum_m(srcs, row_dram):
            """fp8 pair-packed [128,2,M] x2 -> psum colsum -> f32 row -> dram."""
            ps = mps.tile([128, 2048], F32, tag="mm")
            for jc in range(M // 512):
                for p in range(2):
                    nc.tensor.matmul(
                        ps[:, jc * 512:(jc + 1) * 512], lhsT=ones8[:],
                        rhs=srcs[p][:, :, jc * 512:(jc + 1) * 512],
                        start=(p == 0), stop=(p == 1), perf_mode=DR)
            fl = rowp.tile([1, 2048], F32, tag="flm", name="flm", bufs=1)
            nc.vector.tensor_copy(fl[:, :M], ps[0:1, :M])
            nc.sync.dma_start(row_dram.ap(), fl[:, :M])

        def compact_read(row_dram, dst, ps, col0):
            """dram [M] -> [8,128] contiguous read -> PE transpose -> [128,8]."""
            sv = rowp.tile([8, 128], F32, tag="sv", name="sv")
            nc.sync.dma_start(sv[:], row_dram.ap().rearrange("(t p) -> t p", p=128))
            nc.tensor.matmul(ps[:, col0:col0 + NIT], lhsT=sv[:], rhs=id8[:],
                             is_transpose=True)
            nc.vector.tensor_copy(dst[:], ps[:, col0:col0 + NIT])

        # ---------------- prologue ----------------
        for dt in range(KT):
            nc.sync.dma_start(vstage[dt][:],
                              vt.ap()[dt * 128:(dt + 1) * 128, :])
        dma_chunk(0)
        dma_chunk(1)

        # v block: fp8 cast + squares + colsum + compact 1/||v||
        vsq = [sqp.tile([128, 2, M], FP8, tag=f"vsq{p}", bufs=1, name=f"vsq{p}")
               for p in range(2)]
        for dt in range(KT):
            nc.vector.tensor_copy(vb[dt // 2][:, dt % 2, :], vstage[dt][:])
            nc.gpsimd.tensor_tensor(vsq[dt // 2][:, dt % 2, :],
                                    vstage[dt][:], vstage[dt][:], MULT)
        colsum_m(vsq, b_v)
        tp_v = mps.tile([128, 2048], F32, tag="mm")
        compact_read(b_v, ssv_t, tp_v, 0)
        nc.scalar.activation(lssv[:], ssv_t[:], AF.Ln)
        nc.scalar.activation(invv_t[:], lssv[:], AF.Exp, scale=-0.5)
        nc.vector.tensor_scalar(inv2v16[:], invv_t[:], 0.125, None, MULT)

        norm_a(0)
        norm_b(0)
        norm_a(1)
        dma_chunk(2)
        norm_b(1)

        # ---------------- main chunk loop ----------------
        for k in range(NC_CH):
            if k + 2 < NC_CH:
                dma_chunk(k + 2)
            if k == 2:
                for dt in range(KT):
                    nc.sync.dma_start(usstage[dt][:],
                                      us.ap()[dt * 128:(dt + 1) * 128, :])
            for it in range(NIT):
                it_group(k, it)
                if it == 4 and k + 1 < NC_CH:
                    norm_b(k + 1)
                if it == 4 and k == 3:
                    # u-shard ss + diagonal dot (feeds only the epilogue)
                    ussq = [sqp.tile([128, 2, M], FP8, tag=f"ussq{p}", bufs=1, name=f"ussq{p}")
                            for p in range(2)]
                    dpr = [sqp.tile([128, 2, M], FP8, tag=f"dpr{p}", bufs=1, name=f"dpr{p}")
                           for p in range(2)]
                    for dt in range(KT):
                        nc.gpsimd.tensor_tensor(
                            ussq[dt // 2][:, dt % 2, :],
                            usstage[dt][:], usstage[dt][:], MULT)
                        nc.gpsimd.tensor_tensor(
                            dpr[dt // 2][:, dt % 2, :],
                            vstage[dt][:], usstage[dt][:], MULT)
                    colsum_m(ussq, b_us)
                    colsum_m(dpr, b_dot)
                    tp_u = mps.tile([128, 2048], F32, tag="mm")
                    compact_read(b_us, ssus_t, tp_u, 0)
                    compact_read(b_dot, dot_t, tp_u, NIT)
                if it == 4 and k == 4:
                    nc.scalar.activation(lssus[:], ssus_t[:], AF.Ln)
                    nc.scalar.activation(invus_t[:], lssus[:], AF.Exp,
                                         scale=-0.5)
                    s2 = keep.tile([128, NIT], F32)
                    nc.vector.tensor_tensor(s2[:], invv_t[:], invus_t[:], MULT)
                    nc.vector.tensor_scalar(s2[:], s2[:], 2.0, None, MULT)
                    nc.vector.tensor_tensor(e_in[:], dot_t[:], s2[:], MULT)
            if k + 2 < NC_CH:
                norm_a(k + 2)

        # ---------------- epilogue ----------------
        d01 = keep.tile([128, NIT], F32)
        d23 = keep.tile([128, NIT], F32)
        den = keep.tile([128, NIT], F32)
        nc.vector.tensor_tensor(d01[:], dpm[0][:], dpm[1][:], ADD)
        nc.vector.tensor_tensor(d23[:], dpm[2][:], dpm[3][:], ADD)
        nc.vector.tensor_tensor(den[:], d01[:], d23[:], ADD)
        nc.vector.tensor_tensor(den[:], den[:], dpm[4][:], ADD)
        numt = keep.tile([128, NIT], F32)
        nc.scalar.activation(numt[:], e_in[:], AF.Exp)
        nc.vector.tensor_tensor(den[:], den[:], numt[:], ADD)
        lg = keep.tile([128, NIT], F32)
        nc.scalar.activation(lg[:], den[:], AF.Ln)
        lt = keep.tile([128, NIT], F32)
        nc.vector.tensor_tensor(lt[:], lg[:], e_in[:], SUB)
        tp_l = mps.tile([128, 2048], F32, tag="mm")
        nc.tensor.matmul(tp_l[0:NIT, 0:128], lhsT=lt[:], rhs=id128[:],
                         is_transpose=True)
        lout = keep.tile([8, 128], F32)
        nc.vector.tensor_copy(lout[:], tp_l[0:NIT, 0:128])
        nc.sync.dma_start(loss.ap().rearrange("(t p) -> t p", p=128), lout[:])

    # Force Ln/Exp onto the combined natural_log_exp table: the stock
    # chooser greedily picks separate tables for Ln and Exp, inserting a
    # 1.28us ACT_TABLE_LOAD around every pair. Scoped override, restored
    # immediately after compile.
    import concourse.bacc as _bacc_mod
    _orig_tabs = _bacc_mod.get_activation_tables

    def _patched_tabs(arch):
        tabs = {k: set(v) for k, v in _orig_tabs(arch).items()}
        for name, fns in tabs.items():
            if name != "natural_log_exp_and_others":
                fns.discard(AF.Exp)
                fns.discard(AF.Ln)
        return tabs

    _bacc_mod.get_activation_tables = _patched_tabs
    try:
        nc.compile()
    finally:
        _bacc_mod.get_activation_tables = _orig_tabs
    return nc


# ======================================================================
# Host-side entry point: full inputs in, full output out.
# Shards rows of v across the 8 cores; every core gets the full u.
# ======================================================================
import numpy as np

_NC_CACHE = {}


def _get_nc():
    if "nc" not in _NC_CACHE:
        _NC_CACHE["nc"] = build_nc()
    return _NC_CACHE["nc"]


def kernel(v: np.ndarray, u: np.ndarray) -> np.ndarray:
    from concourse.bass_utils import run_bass_kernel_spmd

    nc = _get_nc()
    v = np.asarray(v, dtype=np.float32)
    u = np.asarray(u, dtype=np.float32)
    vT = np.ascontiguousarray(v.T)          # [D, B]
    uT = np.ascontiguousarray(u.T)          # [D, B]
    in_maps = []
    for c in range(NCORES):
        sl = slice(c * M, (c + 1) * M)
        in_maps.append({
            "vt": np.ascontiguousarray(vT[:, sl]),
            "ut": uT,
            "us": np.ascontiguousarray(uT[:, sl]),
        })
    res = run_bass_kernel_spmd(nc, in_maps, core_ids=list(range(NCORES)))
    return np.concatenate([res.results[c]["loss"] for c in range(NCORES)])


# revision 12
# speedup vs baseline: 1.0390x; 1.0390x over previous
"""Bass/Tile kernel for the sharded NT-Xent contrastive loss (streaming fp8).

Per-core computation (core c of 8), B=8192 D=512 M=1024:
  inputs (host pre-transposed, layout only):
    vt [512, 1024] f32 : v[c*M:(c+1)*M, :].T   (this core's v-shard, transposed)
    ut [512, 8192] f32 : u.T                    (full u, transposed)
    us [512, 1024] f32 : u[c*M:(c+1)*M, :].T   (u-shard cols, for the diagonal)
  output:
    loss [1024] f32 : loss rows c*M:(c+1)*M

  math:
    invu[j] = 1/||u_j||  computed as exp(-0.5*ln(ss_u[j]) + ln 16)  (x16 scale
              keeps the fp8 u entries in the normal range)
    un16 = u * invu16  (fp8e4)      vb = v cast fp8e4 (NOT normalized)
    S[i,j] = sum_d vb[d,i]*un16[d,j]   (PE DoubleRow fp8, psum f32, = 16*||v_i||*cos)
    den[i] = sum_j exp((2/(16*||v_i||)) * S[i,j])   (ACT exp accum_out)
    e_in[i] = 2*cos(v_i,u_i) ; loss[i] = ln(exp(e_in) + den) - e_in

u is streamed in column chunks; the chunk norm pipeline (square on Pool,
ones-colsum on PE, compact rsqrt on ACT via ln/exp — single ACT table for the
whole kernel — and a stride-0 broadcast DMA) runs 1-2 chunks ahead of the
matmul+exp pipeline so the ACT engine (the roofline bottleneck at ~55us of
exp work) never idles.
"""

from contextlib import ExitStack
from math import log

import concourse.bass as bass
import concourse.tile as tile
from concourse import masks
from concourse import bacc, mybir

F32 = mybir.dt.float32
BF16 = mybir.dt.bfloat16
FP8 = mybir.dt.float8e4
MULT = mybir.AluOpType.mult
ADD = mybir.AluOpType.add
SUB = mybir.AluOpType.subtract
AF = mybir.ActivationFunctionType
DR = mybir.MatmulPerfMode.DoubleRow

B = 8192
D = 512
NCORES = 8
M = B // NCORES   # 1024
KT = D // 128     # 4 d-tiles
NIT = M // 128    # 8 i-tiles
LN16 = log(16.0)

CH = [1024, 1024, 2048, 2048, 2048]          # u column chunk sizes
OFF = [0, 1024, 2048, 4096, 6144]            # chunk column offsets
NC_CH = len(CH)


def build_nc():
    nc = bacc.Bacc("TRN2", target_bir_lowering=False, debug=False,
                   num_devices=NCORES)

    vt = nc.dram_tensor("vt", [D, M], F32, kind="ExternalInput")
    ut = nc.dram_tensor("ut", [D, B], F32, kind="ExternalInput")
    us = nc.dram_tensor("us", [D, M], F32, kind="ExternalInput")
    loss = nc.dram_tensor("loss", [M], F32, kind="ExternalOutput")
    # DRAM bounce buffers for transposes / broadcasts
    b_ss = nc.dram_tensor("b_ss", [B], BF16)     # u sum-of-squares rows
    b_inv = nc.dram_tensor("b_inv", [B], FP8)    # 16/||u_j|| flat
    b_v = nc.dram_tensor("b_v", [M], F32)       # v ss row
    b_us = nc.dram_tensor("b_us", [M], F32)     # u-shard ss row
    b_dot = nc.dram_tensor("b_dot", [M], F32)   # v.u dot row

    with tile.TileContext(nc) as tc, ExitStack() as ctx:
        consts = ctx.enter_context(tc.tile_pool(name="consts", bufs=1))
        keep = ctx.enter_context(tc.tile_pool(name="keep", bufs=1))
        upool = ctx.enter_context(tc.tile_pool(name="upool", bufs=1))
        stage = ctx.enter_context(tc.tile_pool(name="stage", bufs=2))
        vst = ctx.enter_context(tc.tile_pool(name="vst", bufs=1))
        sqp = ctx.enter_context(tc.tile_pool(name="sqp", bufs=2))
        rowp = ctx.enter_context(tc.tile_pool(name="rowp", bufs=2))
        invp = ctx.enter_context(tc.tile_pool(name="invp", bufs=2))
        expool = ctx.enter_context(tc.tile_pool(name="expool", bufs=1))
        mps = ctx.enter_context(tc.tile_pool(name="mps", bufs=2, space="PSUM"))

        # all-ones fp8 stationary operand: DoubleRow colsum of rhs
        ones8 = consts.tile([128, 2, 128], FP8)
        nc.vector.memset(ones8[:], 1.0)
        bias16 = consts.tile([128, 1], F32)
        nc.vector.memset(bias16[:], LN16)
        id8 = consts.tile([8, 8], F32)
        masks.make_identity(nc, id8[:])
        id128 = consts.tile([128, 128], F32)
        masks.make_identity(nc, id128[:])

        # persistent tiles
        un = [keep.tile([128, 2, B], FP8, tag=f"un{p}", name=f"un{p}")
              for p in range(2)]                 # normalized*16 u, kpair-packed
        vb = [keep.tile([128, 2, M], FP8, tag=f"vb{p}", name=f"vb{p}")
              for p in range(2)]                 # raw v fp8, kpair-packed
        inv2v16 = keep.tile([128, NIT], F32)     # 2/(16*||v_i||)
        e_in = keep.tile([128, NIT], F32)        # 2*cos(v_i, u_i)
        dpm = [keep.tile([128, NIT], F32, tag=f"dpm{k}", name=f"dpm{k}")
               for k in range(NC_CH)]            # per-chunk exp row sums
        invv_t = keep.tile([128, NIT], F32)
        invus_t = keep.tile([128, NIT], F32)
        ssv_t = keep.tile([128, NIT], F32)
        ssus_t = keep.tile([128, NIT], F32)
        dot_t = keep.tile([128, NIT], F32)
        lssv = keep.tile([128, NIT], F32)
        lssus = keep.tile([128, NIT], F32)

        vstage = [vst.tile([128, M], F32, tag=f"vs{dt}", name=f"vs{dt}")
                  for dt in range(KT)]
        usstage = [vst.tile([128, M], F32, tag=f"us{dt}", name=f"us{dt}")
                   for dt in range(KT)]

        # ---------------- emission helpers ----------------
        def dma_chunk(k):
            c, j0 = CH[k], OFF[k]
            for dt in range(KT):
                t = stage.tile([128, 2048], F32, tag=f"ust{dt}", bufs=3)
                nc.sync.dma_start(
                    t[:, :c], ut.ap()[dt * 128:(dt + 1) * 128, j0:j0 + c])
                stage_t[k][dt] = t

        stage_t = [[None] * KT for _ in range(NC_CH)]
        sc_t = [None] * NC_CH    # compact ss [128, 16] bf16 per chunk
        invc_t = [None] * NC_CH  # compact invu16 fp8 per chunk

        def norm_a(k):
            """square -> colsum -> row bounce -> compact ss (no ACT)."""
            c, j0 = CH[k], OFF[k]
            sq = [sqp.tile([128, 2, 2048], FP8, tag=f"usq{p}", name=f"usq{p}", bufs=1) for p in range(2)]
            for dt in range(KT):
                st = stage_t[k][dt]
                nc.gpsimd.tensor_tensor(sq[dt // 2][:, dt % 2, :c],
                                        st[:, :c], st[:, :c], MULT)
            ps = mps.tile([128, 2048], F32, tag="mm")
            for jc in range(c // 512):
                for p in range(2):
                    nc.tensor.matmul(
                        ps[:, jc * 512:(jc + 1) * 512], lhsT=ones8[:],
                        rhs=sq[p][:, :, jc * 512:(jc + 1) * 512],
                        start=(p == 0), stop=(p == 1), perf_mode=DR)
            sc_t[k] = ps

        def norm_b(k):
            """compact rsqrt (ACT) -> flat bounce -> bcast dma -> fp8 cast."""
            c, j0 = CH[k], OFF[k]
            ps = sc_t[k]
            nc.scalar.activation(ps[:, :c], ps[:, :c], AF.Ln)
            invu = invp.tile([128, 2048], FP8, tag="invu")
            nc.scalar.activation(invu[:, :c], ps[:, :c], AF.Exp,
                                 scale=-0.5, bias=bias16[:])
            for dt in range(KT):
                nc.vector.tensor_tensor(
                    un[dt // 2][:, dt % 2, j0:j0 + c],
                    stage_t[k][dt][:, :c], invu[:, :c], MULT)

        def it_group(k, it):
            c, j0 = CH[k], OFF[k]
            ps = mps.tile([128, 2048], F32, tag="mm")
            for p in range(2):
                for jc in range(c // 512):
                    nc.tensor.matmul(
                        ps[:, jc * 512:(jc + 1) * 512],
                        lhsT=vb[p][:, :, it * 128:(it + 1) * 128],
                        rhs=un[p][:, :, j0 + jc * 512:j0 + (jc + 1) * 512],
                        start=(p == 0), stop=(p == 1), perf_mode=DR)
            ex = expool.tile([128, 2048], FP8, tag="ex")
            nc.scalar.activation(ex[:, :c], ps[:, :c], AF.Exp,
                                 scale=inv2v16[:, it:it + 1],
                                 accum_out=dpm[k][:, it:it + 1])

        def colsum_m(srcs, row_dram):
            """fp8 pair-packed [128,2,M] x2 -> psum colsum -> f32 row -> dram."""
            ps = mps.tile([128, 2048], F32, tag="mm")
            for jc in range(M // 512):
                for p in range(2):
                    nc.tensor.matmul(
                        ps[:, jc * 512:(jc + 1) * 512], lhsT=ones8[:],
                        rhs=srcs[p][:, :, jc * 512:(jc + 1) * 512],
                        start=(p == 0), stop=(p == 1), perf_mode=DR)
            fl = rowp.tile([1, 2048], F32, tag="flm", name="flm", bufs=1)
            nc.vector.tensor_copy(fl[:, :M], ps[0:1, :M])
            nc.sync.dma_start(row_dram.ap(), fl[:, :M])

        def compact_read(row_dram, dst, ps, col0):
            """dram [M] -> [8,128] contiguous read -> PE transpose -> [128,8]."""
            sv = rowp.tile([8, 128], F32, tag="sv", name="sv")
            nc.sync.dma_start(sv[:], row_dram.ap().rearrange("(t p) -> t p", p=128))
            nc.tensor.matmul(ps[:, col0:col0 + NIT], lhsT=sv[:], rhs=id8[:],
                             is_transpose=True)
            nc.vector.tensor_copy(dst[:], ps[:, col0:col0 + NIT])

        # ---------------- prologue ----------------
        for dt in range(KT):
            nc.sync.dma_start(vstage[dt][:],
                              vt.ap()[dt * 128:(dt + 1) * 128, :])
        dma_chunk(0)
        dma_chunk(1)

        # v block: fp8 cast + squares + colsum + compact 1/||v||
        vsq = [sqp.tile([128, 2, M], FP8, tag=f"vsq{p}", bufs=1, name=f"vsq{p}")
               for p in range(2)]
        for dt in range(KT):
            nc.vector.tensor_copy(vb[dt // 2][:, dt % 2, :], vstage[dt][:])
            nc.gpsimd.tensor_tensor(vsq[dt // 2][:, dt % 2, :],
                                    vstage[dt][:], vstage[dt][:], MULT)
        colsum_m(vsq, b_v)
        tp_v = mps.tile([128, 2048], F32, tag="mm")
        compact_read(b_v, ssv_t, tp_v, 0)
        nc.scalar.activation(lssv[:], ssv_t[:], AF.Ln)
        nc.scalar.activation(invv_t[:], lssv[:], AF.Exp, scale=-0.5)
        nc.vector.tensor_scalar(inv2v16[:], invv_t[:], 0.125, None, MULT)

        norm_a(0)
        norm_b(0)
        norm_a(1)
        dma_chunk(2)
        norm_b(1)

        # ---------------- main chunk loop ----------------
        for k in range(NC_CH):
            if k + 2 < NC_CH:
                dma_chunk(k + 2)
            if k == 2:
                for dt in range(KT):
                    nc.sync.dma_start(usstage[dt][:],
                                      us.ap()[dt * 128:(dt + 1) * 128, :])
            for it in range(NIT):
                it_group(k, it)
                if it == 4 and k + 1 < NC_CH:
                    norm_a(k + 1)
                    norm_b(k + 1)
                if it == 4 and k == 3:
                    # u-shard ss + diagonal dot (feeds only the epilogue)
                    ussq = [sqp.tile([128, 2, M], FP8, tag=f"ussq{p}", bufs=1, name=f"ussq{p}")
                            for p in range(2)]
                    dpr = [sqp.tile([128, 2, M], FP8, tag=f"dpr{p}", bufs=1, name=f"dpr{p}")
                           for p in range(2)]
                    for dt in range(KT):
                        nc.gpsimd.tensor_tensor(
                            ussq[dt // 2][:, dt % 2, :],
                            usstage[dt][:], usstage[dt][:], MULT)
                        nc.gpsimd.tensor_tensor(
                            dpr[dt // 2][:, dt % 2, :],
                            vstage[dt][:], usstage[dt][:], MULT)
                    colsum_m(ussq, b_us)
                    colsum_m(dpr, b_dot)
                    tp_u = mps.tile([128, 2048], F32, tag="mm")
                    compact_read(b_us, ssus_t, tp_u, 0)
                    compact_read(b_dot, dot_t, tp_u, NIT)
                if it == 4 and k == 4:
                    nc.scalar.activation(lssus[:], ssus_t[:], AF.Ln)
                    nc.scalar.activation(invus_t[:], lssus[:], AF.Exp,
                                         scale=-0.5)
                    s2 = keep.tile([128, NIT], F32)
                    nc.vector.tensor_tensor(s2[:], invv_t[:], invus_t[:], MULT)
                    nc.vector.tensor_scalar(s2[:], s2[:], 2.0, None, MULT)
                    nc.vector.tensor_tensor(e_in[:], dot_t[:], s2[:], MULT)

        # ---------------- epilogue ----------------
        d01 = keep.tile([128, NIT], F32)
        d23 = keep.tile([128, NIT], F32)
        den = keep.tile([128, NIT], F32)
        nc.vector.tensor_tensor(d01[:], dpm[0][:], dpm[1][:], ADD)
        nc.vector.tensor_tensor(d23[:], dpm[2][:], dpm[3][:], ADD)
        nc.vector.tensor_tensor(den[:], d01[:], d23[:], ADD)
        nc.vector.tensor_tensor(den[:], den[:], dpm[4][:], ADD)
        numt = keep.tile([128, NIT], F32)
        nc.scalar.activation(numt[:], e_in[:], AF.Exp)
        nc.vector.tensor_tensor(den[:], den[:], numt[:], ADD)
        lg = keep.tile([128, NIT], F32)
        nc.scalar.activation(lg[:], den[:], AF.Ln)
        lt = keep.tile([128, NIT], F32)
        nc.vector.tensor_tensor(lt[:], lg[:], e_in[:], SUB)
        tp_l = mps.tile([128, 2048], F32, tag="mm")
        nc.tensor.matmul(tp_l[0:NIT, 0:128], lhsT=lt[:], rhs=id128[:],
                         is_transpose=True)
        lout = keep.tile([8, 128], F32)
        nc.vector.tensor_copy(lout[:], tp_l[0:NIT, 0:128])
        nc.sync.dma_start(loss.ap().rearrange("(t p) -> t p", p=128), lout[:])

    # Force Ln/Exp onto the combined natural_log_exp table: the stock
    # chooser greedily picks separate tables for Ln and Exp, inserting a
    # 1.28us ACT_TABLE_LOAD around every pair. Scoped override, restored
    # immediately after compile.
    import concourse.bacc as _bacc_mod
    _orig_tabs = _bacc_mod.get_activation_tables

    def _patched_tabs(arch):
        tabs = {k: set(v) for k, v in _orig_tabs(arch).items()}
        for name, fns in tabs.items():
            if name != "natural_log_exp_and_others":
                fns.discard(AF.Exp)
                fns.discard(AF.Ln)
        return tabs

    _bacc_mod.get_activation_tables = _patched_tabs
    try:
        nc.compile()
    finally:
        _bacc_mod.get_activation_tables = _orig_tabs
    return nc


# ======================================================================
# Host-side entry point: full inputs in, full output out.
# Shards rows of v across the 8 cores; every core gets the full u.
# ======================================================================
import numpy as np

_NC_CACHE = {}


def _get_nc():
    if "nc" not in _NC_CACHE:
        _NC_CACHE["nc"] = build_nc()
    return _NC_CACHE["nc"]


def kernel(v: np.ndarray, u: np.ndarray) -> np.ndarray:
    from concourse.bass_utils import run_bass_kernel_spmd

    nc = _get_nc()
    v = np.asarray(v, dtype=np.float32)
    u = np.asarray(u, dtype=np.float32)
    vT = np.ascontiguousarray(v.T)          # [D, B]
    uT = np.ascontiguousarray(u.T)          # [D, B]
    in_maps = []
    for c in range(NCORES):
        sl = slice(c * M, (c + 1) * M)
        in_maps.append({
            "vt": np.ascontiguousarray(vT[:, sl]),
            "ut": uT,
            "us": np.ascontiguousarray(uT[:, sl]),
        })
    res = run_bass_kernel_spmd(nc, in_maps, core_ids=list(range(NCORES)))
    return np.concatenate([res.results[c]["loss"] for c in range(NCORES)])


# revision 13
# speedup vs baseline: 1.0795x; 1.0389x over previous
"""Bass/Tile kernel for the sharded NT-Xent contrastive loss (streaming fp8).

Per-core computation (core c of 8), B=8192 D=512 M=1024:
  inputs (host pre-transposed, layout only):
    vt [512, 1024] f32 : v[c*M:(c+1)*M, :].T   (this core's v-shard, transposed)
    ut [512, 8192] f32 : u.T                    (full u, transposed)
    us [512, 1024] f32 : u[c*M:(c+1)*M, :].T   (u-shard cols, for the diagonal)
  output:
    loss [1024] f32 : loss rows c*M:(c+1)*M

  math:
    invu[j] = 1/||u_j||  computed as exp(-0.5*ln(ss_u[j]) + ln 16)  (x16 scale
              keeps the fp8 u entries in the normal range)
    un16 = u * invu16  (fp8e4)      vb = v cast fp8e4 (NOT normalized)
    S[i,j] = sum_d vb[d,i]*un16[d,j]   (PE DoubleRow fp8, psum f32, = 16*||v_i||*cos)
    den[i] = sum_j exp((2/(16*||v_i||)) * S[i,j])   (ACT exp accum_out)
    e_in[i] = 2*cos(v_i,u_i) ; loss[i] = ln(exp(e_in) + den) - e_in

u is streamed in column chunks; the chunk norm pipeline (square on Pool,
ones-colsum on PE, compact rsqrt on ACT via ln/exp — single ACT table for the
whole kernel — and a stride-0 broadcast DMA) runs 1-2 chunks ahead of the
matmul+exp pipeline so the ACT engine (the roofline bottleneck at ~55us of
exp work) never idles.
"""

from contextlib import ExitStack
from math import log

import concourse.bass as bass
import concourse.tile as tile
from concourse import masks
from concourse import bacc, mybir

F32 = mybir.dt.float32
BF16 = mybir.dt.bfloat16
FP8 = mybir.dt.float8e4
MULT = mybir.AluOpType.mult
ADD = mybir.AluOpType.add
SUB = mybir.AluOpType.subtract
AF = mybir.ActivationFunctionType
DR = mybir.MatmulPerfMode.DoubleRow

B = 8192
D = 512
NCORES = 8
M = B // NCORES   # 1024
KT = D // 128     # 4 d-tiles
NIT = M // 128    # 8 i-tiles
LN16 = log(16.0)

CH = [1024, 1024, 2048, 2048, 2048]          # u column chunk sizes
OFF = [0, 1024, 2048, 4096, 6144]            # chunk column offsets
NC_CH = len(CH)


def build_nc():
    nc = bacc.Bacc("TRN2", target_bir_lowering=False, debug=False,
                   num_devices=NCORES)

    vt = nc.dram_tensor("vt", [D, M], F32, kind="ExternalInput")
    ut = nc.dram_tensor("ut", [D, B], F32, kind="ExternalInput")
    us = nc.dram_tensor("us", [D, M], F32, kind="ExternalInput")
    loss = nc.dram_tensor("loss", [M], F32, kind="ExternalOutput")
    # DRAM bounce buffers for transposes / broadcasts
    b_ss = nc.dram_tensor("b_ss", [B], BF16)     # u sum-of-squares rows
    b_inv = nc.dram_tensor("b_inv", [B], FP8)    # 16/||u_j|| flat
    b_v = nc.dram_tensor("b_v", [M], F32)       # v ss row
    b_us = nc.dram_tensor("b_us", [M], F32)     # u-shard ss row
    b_dot = nc.dram_tensor("b_dot", [M], F32)   # v.u dot row

    with tile.TileContext(nc) as tc, ExitStack() as ctx:
        consts = ctx.enter_context(tc.tile_pool(name="consts", bufs=1))
        keep = ctx.enter_context(tc.tile_pool(name="keep", bufs=1))
        upool = ctx.enter_context(tc.tile_pool(name="upool", bufs=1))
        stage = ctx.enter_context(tc.tile_pool(name="stage", bufs=2))
        vst = ctx.enter_context(tc.tile_pool(name="vst", bufs=1))
        sqp = ctx.enter_context(tc.tile_pool(name="sqp", bufs=2))
        rowp = ctx.enter_context(tc.tile_pool(name="rowp", bufs=2))
        invp = ctx.enter_context(tc.tile_pool(name="invp", bufs=2))
        expool = ctx.enter_context(tc.tile_pool(name="expool", bufs=1))
        mps = ctx.enter_context(tc.tile_pool(name="mps", bufs=4, space="PSUM"))

        # all-ones fp8 stationary operand: DoubleRow colsum of rhs
        ones8 = consts.tile([128, 2, 128], FP8)
        nc.vector.memset(ones8[:], 1.0)
        bias16 = consts.tile([128, 1], F32)
        nc.vector.memset(bias16[:], LN16)
        id8 = consts.tile([8, 8], F32)
        masks.make_identity(nc, id8[:])
        id128 = consts.tile([128, 128], F32)
        masks.make_identity(nc, id128[:])

        # persistent tiles
        un = [keep.tile([128, 2, B], FP8, tag=f"un{p}", name=f"un{p}")
              for p in range(2)]                 # normalized*16 u, kpair-packed
        vb = [keep.tile([128, 2, M], FP8, tag=f"vb{p}", name=f"vb{p}")
              for p in range(2)]                 # raw v fp8, kpair-packed
        inv2v16 = keep.tile([128, NIT], F32)     # 2/(16*||v_i||)
        e_in = keep.tile([128, NIT], F32)        # 2*cos(v_i, u_i)
        HB = [0, 1, 2, 4, 6]                     # chunk -> first half index
        NH = 8
        dpm = [keep.tile([128, NIT], F32, tag=f"dpm{k}", name=f"dpm{k}")
               for k in range(NH)]               # per-chunk-half exp row sums
        invv_t = keep.tile([128, NIT], F32)
        invus_t = keep.tile([128, NIT], F32)
        ssv_t = keep.tile([128, NIT], F32)
        ssus_t = keep.tile([128, NIT], F32)
        dot_t = keep.tile([128, NIT], F32)
        lssv = keep.tile([128, NIT], F32)
        lssus = keep.tile([128, NIT], F32)

        vstage = [vst.tile([128, M], F32, tag=f"vs{dt}", name=f"vs{dt}")
                  for dt in range(KT)]
        usstage = [vst.tile([128, M], F32, tag=f"us{dt}", name=f"us{dt}")
                   for dt in range(KT)]

        # ---------------- emission helpers ----------------
        def dma_chunk(k):
            c, j0 = CH[k], OFF[k]
            for dt in range(KT):
                t = stage.tile([128, 2048], F32, tag=f"ust{dt}", bufs=3)
                nc.sync.dma_start(
                    t[:, :c], ut.ap()[dt * 128:(dt + 1) * 128, j0:j0 + c])
                stage_t[k][dt] = t

        stage_t = [[None] * KT for _ in range(NC_CH)]
        sc_t = [None] * NC_CH    # compact ss [128, 16] bf16 per chunk
        invc_t = [None] * NC_CH  # compact invu16 fp8 per chunk

        def norm_a(k):
            """square -> colsum -> row bounce -> compact ss (no ACT)."""
            c, j0 = CH[k], OFF[k]
            sq = [sqp.tile([128, 2, 2048], FP8, tag=f"usq{p}", name=f"usq{p}", bufs=1) for p in range(2)]
            for dt in range(KT):
                st = stage_t[k][dt]
                nc.gpsimd.tensor_tensor(sq[dt // 2][:, dt % 2, :c],
                                        st[:, :c], st[:, :c], MULT)
            pss = []
            for h in range(c // 1024):
                ps = mps.tile([128, 1024], F32, tag="mm")
                for jc in range(2):
                    for p in range(2):
                        j5 = h * 1024 + jc * 512
                        nc.tensor.matmul(
                            ps[:, jc * 512:(jc + 1) * 512], lhsT=ones8[:],
                            rhs=sq[p][:, :, j5:j5 + 512],
                            start=(p == 0), stop=(p == 1), perf_mode=DR)
                pss.append(ps)
            sc_t[k] = pss

        def norm_b(k):
            """compact rsqrt (ACT) -> flat bounce -> bcast dma -> fp8 cast."""
            c, j0 = CH[k], OFF[k]
            invu = invp.tile([128, 2048], FP8, tag="invu")
            for h, ps in enumerate(sc_t[k]):
                nc.scalar.activation(ps[:], ps[:], AF.Ln)
                nc.scalar.activation(invu[:, h * 1024:(h + 1) * 1024], ps[:],
                                     AF.Exp, scale=-0.5, bias=bias16[:])
            for dt in range(KT):
                nc.vector.tensor_tensor(
                    un[dt // 2][:, dt % 2, j0:j0 + c],
                    stage_t[k][dt][:, :c], invu[:, :c], MULT)

        def it_group(k, it):
            c, j0 = CH[k], OFF[k]
            for h in range(c // 1024):
                ps = mps.tile([128, 1024], F32, tag="mm")
                for p in range(2):
                    for jc in range(2):
                        j5 = j0 + h * 1024 + jc * 512
                        nc.tensor.matmul(
                            ps[:, jc * 512:(jc + 1) * 512],
                            lhsT=vb[p][:, :, it * 128:(it + 1) * 128],
                            rhs=un[p][:, :, j5:j5 + 512],
                            start=(p == 0), stop=(p == 1), perf_mode=DR)
                ex = expool.tile([128, 2048], FP8, tag="ex")
                nc.scalar.activation(ex[:, :1024], ps[:], AF.Exp,
                                     scale=inv2v16[:, it:it + 1],
                                     accum_out=dpm[HB[k] + h][:, it:it + 1])

        def colsum_m(srcs, row_dram):
            """fp8 pair-packed [128,2,M] x2 -> psum colsum -> f32 row -> dram."""
            ps = mps.tile([128, 1024], F32, tag="mm")
            for jc in range(M // 512):
                for p in range(2):
                    nc.tensor.matmul(
                        ps[:, jc * 512:(jc + 1) * 512], lhsT=ones8[:],
                        rhs=srcs[p][:, :, jc * 512:(jc + 1) * 512],
                        start=(p == 0), stop=(p == 1), perf_mode=DR)
            fl = rowp.tile([1, 2048], F32, tag="flm", name="flm", bufs=1)
            nc.vector.tensor_copy(fl[:, :M], ps[0:1, :M])
            nc.sync.dma_start(row_dram.ap(), fl[:, :M])

        def compact_read(row_dram, dst, ps, col0):
            """dram [M] -> [8,128] contiguous read -> PE transpose -> [128,8]."""
            sv = rowp.tile([8, 128], F32, tag="sv", name="sv")
            nc.sync.dma_start(sv[:], row_dram.ap().rearrange("(t p) -> t p", p=128))
            nc.tensor.matmul(ps[:, col0:col0 + NIT], lhsT=sv[:], rhs=id8[:],
                             is_transpose=True)
            nc.vector.tensor_copy(dst[:], ps[:, col0:col0 + NIT])

        # ---------------- prologue ----------------
        for dt in range(KT):
            nc.sync.dma_start(vstage[dt][:],
                              vt.ap()[dt * 128:(dt + 1) * 128, :])
        dma_chunk(0)
        dma_chunk(1)

        # v block: fp8 cast + squares + colsum + compact 1/||v||
        vsq = [sqp.tile([128, 2, M], FP8, tag=f"vsq{p}", bufs=1, name=f"vsq{p}")
               for p in range(2)]
        for dt in range(KT):
            nc.vector.tensor_copy(vb[dt // 2][:, dt % 2, :], vstage[dt][:])
            nc.gpsimd.tensor_tensor(vsq[dt // 2][:, dt % 2, :],
                                    vstage[dt][:], vstage[dt][:], MULT)
        colsum_m(vsq, b_v)
        tp_v = mps.tile([128, 1024], F32, tag="mm")
        compact_read(b_v, ssv_t, tp_v, 0)
        nc.scalar.activation(lssv[:], ssv_t[:], AF.Ln)
        nc.scalar.activation(invv_t[:], lssv[:], AF.Exp, scale=-0.5)
        nc.vector.tensor_scalar(inv2v16[:], invv_t[:], 0.125, None, MULT)

        norm_a(0)
        norm_b(0)
        norm_a(1)
        dma_chunk(2)
        norm_b(1)

        # ---------------- main chunk loop ----------------
        for k in range(NC_CH):
            if k + 2 < NC_CH:
                dma_chunk(k + 2)
            if k == 2:
                for dt in range(KT):
                    nc.sync.dma_start(usstage[dt][:],
                                      us.ap()[dt * 128:(dt + 1) * 128, :])
            for it in range(NIT):
                it_group(k, it)
                if it == 4 and k + 1 < NC_CH:
                    norm_a(k + 1)
                    norm_b(k + 1)
                if it == 4 and k == 3:
                    # u-shard ss + diagonal dot (feeds only the epilogue)
                    ussq = [sqp.tile([128, 2, M], FP8, tag=f"ussq{p}", bufs=1, name=f"ussq{p}")
                            for p in range(2)]
                    dpr = [sqp.tile([128, 2, M], FP8, tag=f"dpr{p}", bufs=1, name=f"dpr{p}")
                           for p in range(2)]
                    for dt in range(KT):
                        nc.gpsimd.tensor_tensor(
                            ussq[dt // 2][:, dt % 2, :],
                            usstage[dt][:], usstage[dt][:], MULT)
                        nc.gpsimd.tensor_tensor(
                            dpr[dt // 2][:, dt % 2, :],
                            vstage[dt][:], usstage[dt][:], MULT)
                    colsum_m(ussq, b_us)
                    colsum_m(dpr, b_dot)
                    tp_u = mps.tile([128, 1024], F32, tag="mm")
                    compact_read(b_us, ssus_t, tp_u, 0)
                    compact_read(b_dot, dot_t, tp_u, NIT)
                if it == 4 and k == 4:
                    nc.scalar.activation(lssus[:], ssus_t[:], AF.Ln)
                    nc.scalar.activation(invus_t[:], lssus[:], AF.Exp,
                                         scale=-0.5)
                    s2 = keep.tile([128, NIT], F32)
                    nc.vector.tensor_tensor(s2[:], invv_t[:], invus_t[:], MULT)
                    nc.vector.tensor_scalar(s2[:], s2[:], 2.0, None, MULT)
                    nc.vector.tensor_tensor(e_in[:], dot_t[:], s2[:], MULT)

        # ---------------- epilogue ----------------
        d01 = keep.tile([128, NIT], F32)
        d23 = keep.tile([128, NIT], F32)
        den = keep.tile([128, NIT], F32)
        nc.vector.tensor_tensor(d01[:], dpm[0][:], dpm[1][:], ADD)
        nc.vector.tensor_tensor(d23[:], dpm[2][:], dpm[3][:], ADD)
        nc.vector.tensor_tensor(den[:], d01[:], d23[:], ADD)
        for hx in range(4, NH):
            nc.vector.tensor_tensor(den[:], den[:], dpm[hx][:], ADD)
        numt = keep.tile([128, NIT], F32)
        nc.scalar.activation(numt[:], e_in[:], AF.Exp)
        nc.vector.tensor_tensor(den[:], den[:], numt[:], ADD)
        lg = keep.tile([128, NIT], F32)
        nc.scalar.activation(lg[:], den[:], AF.Ln)
        lt = keep.tile([128, NIT], F32)
        nc.vector.tensor_tensor(lt[:], lg[:], e_in[:], SUB)
        tp_l = mps.tile([128, 1024], F32, tag="mm")
        nc.tensor.matmul(tp_l[0:NIT, 0:128], lhsT=lt[:], rhs=id128[:],
                         is_transpose=True)
        lout = keep.tile([8, 128], F32)
        nc.vector.tensor_copy(lout[:], tp_l[0:NIT, 0:128])
        nc.sync.dma_start(loss.ap().rearrange("(t p) -> t p", p=128), lout[:])

    # Force Ln/Exp onto the combined natural_log_exp table: the stock
    # chooser greedily picks separate tables for Ln and Exp, inserting a
    # 1.28us ACT_TABLE_LOAD around every pair. Scoped override, restored
    # immediately after compile.
    import concourse.bacc as _bacc_mod
    _orig_tabs = _bacc_mod.get_activation_tables

    def _patched_tabs(arch):
        tabs = {k: set(v) for k, v in _orig_tabs(arch).items()}
        for name, fns in tabs.items():
            if name != "natural_log_exp_and_others":
                fns.discard(AF.Exp)
                fns.discard(AF.Ln)
        return tabs

    _bacc_mod.get_activation_tables = _patched_tabs
    try:
        nc.compile()
    finally:
        _bacc_mod.get_activation_tables = _orig_tabs
    return nc


# ======================================================================
# Host-side entry point: full inputs in, full output out.
# Shards rows of v across the 8 cores; every core gets the full u.
# ======================================================================
import numpy as np

_NC_CACHE = {}


def _get_nc():
    if "nc" not in _NC_CACHE:
        _NC_CACHE["nc"] = build_nc()
    return _NC_CACHE["nc"]


def kernel(v: np.ndarray, u: np.ndarray) -> np.ndarray:
    from concourse.bass_utils import run_bass_kernel_spmd

    nc = _get_nc()
    v = np.asarray(v, dtype=np.float32)
    u = np.asarray(u, dtype=np.float32)
    vT = np.ascontiguousarray(v.T)          # [D, B]
    uT = np.ascontiguousarray(u.T)          # [D, B]
    in_maps = []
    for c in range(NCORES):
        sl = slice(c * M, (c + 1) * M)
        in_maps.append({
            "vt": np.ascontiguousarray(vT[:, sl]),
            "ut": uT,
            "us": np.ascontiguousarray(uT[:, sl]),
        })
    res = run_bass_kernel_spmd(nc, in_maps, core_ids=list(range(NCORES)))
    return np.concatenate([res.results[c]["loss"] for c in range(NCORES)])
